# revision 28
# baseline (speedup 1.0000x reference)
"""Bidirectional S6 (Mamba-style) cross-modal fusion block on 8 Trainium2 cores.

Self-contained: hardcodes shapes from the problem spec.
  x, y: (8, 128, 64, 64) f32 -> out: (8, 128, 64, 64) f32

Sharding: data-parallel over batch (8 batches -> 8 cores). Each core runs both
scan directions for its batch; no collectives.

Per-core algorithm (L = 64*64 = 4096, d_model = 64, d_state = 64), with the
forward/reverse directions packed into partitions 0:64 / 64:128 of shared
tiles (reverse-direction tensors stored time-reversed so both scans run
"forward"):
  xs = to_x_w @ x + b                     (PE)
  ys = to_y_w @ y + b                     (PE)
  u  = silu(depthwise_conv4(xs))          (PE diag-matmuls + ACT sigmoid + DVE)
  dt = softplus(dt_w @ (Wdt @ ys) + b)    (PE + ACT exp/ln)
  B, C = WB @ ys, WC @ ys                 (PE, stored stacked x2 as 128 rows)
  w  = dt * u                             (DVE)
  scan over 32 tiles of 128 partitions = (2 d-channels x 64 states):
    dt_b = selector_j @ dt                (PE broadcast into (d,n) layout)
    dA   = exp(A_scale_j * dt_b)          (ACT, per-partition scale, f32)
    w_b  = selector_j @ w                 (PE)
    dBu  = w_b * B2                       (DVE, bf16 2x)
    h    = scan(h = dA*h + dBu)           (DVE tensor_tensor_scan, f32 state)
    hC   = h * C2                         (DVE, bf16 2x)
    o   += P64_j @ hC                     (PE partition-reduce, PSUM accum)
  o += diag(D) @ u                        (PE)
  out = proj_w @ concat(oF, rev(oR)) + b  (PE)
"""

import numpy as np

# ---------------------------------------------------------------- config
DIM = 128
DM = 64          # d_model per branch
NS = 64          # d_state
NT = 32          # partition tiles: 2 d-channels x 64 states each
DCONV = 4
DTRANK = 4
B_TOT = 8
H = W = 64
L = H * W        # 4096

CFG = {
    "NM": 2,              # megachunks along L for the scan
    "DT_STREAM": "bf16",  # dtype of B2/C2/dBu/h/hC stream ("f32" | "bf16")
    "DT_DA": "f32",       # dtype of dA (decay factors)
    "TTS_POOL": 0,        # scan tiles on GpSimd (walrus rejects: keep 0)
    "HC_POOL": 6,         # of every 6 scan tiles, how many run hC-mul on GpSimd
    "BUFS": 3,            # sbuf stream pool bufs
    "NWD": 1,             # broadcast-DMA splits per 64-row half
}

_BUILD_CACHE = {}
LAST_RESULT = None


def _dt(name):
    import concourse.mybir as mybir
    return {"f32": mybir.dt.float32, "bf16": mybir.dt.bfloat16}[name]


def _np_dt(name):
    import ml_dtypes
    return {"f32": np.float32, "bf16": ml_dtypes.bfloat16}[name]


def build_program(cfg, L=L):
    """Build the per-core Bass program. Returns nc."""
    import concourse.mybir as mybir
    import concourse.tile as tile
    from concourse import bacc

    f32 = mybir.dt.float32
    DT_S = _dt(cfg["DT_STREAM"])

    nc = bacc.Bacc()

    dram = {}

    def din(name, shape, dtype=f32):
        dram[name] = nc.dram_tensor(name, list(shape), dtype, kind="ExternalInput")

    din("x", (DIM, L))
    din("y", (DIM, L))
    din("toxwT", (DIM, DM))
    din("toxb", (DM, 1))
    din("toywT", (DIM, DIM))
    din("toyb", (DIM, 1))
    din("projwT", (DIM, DIM), DT_S)
    din("projb", (DIM, 1))
    din("selJ2", (DIM, NT, 128), DT_S)
    din("P64", (DIM, NT, DM), DT_S)
    din("dtwT2", (68, DM))
    din("convdiag2", (DIM, DCONV, DM))
    din("Ddiag2", (DIM, DM), DT_S)
    for p in ("f", "r"):
        din(p + "WdtT", (DIM, DTRANK))
        din(p + "WBT", (DIM, NS))
        din(p + "WCT", (DIM, NS))
        din(p + "dtb", (DM, 1))
        din(p + "convb", (DM, 1))
        din(p + "Ascale", (DIM, NT))

    out_d = nc.dram_tensor("out", [DIM, L], f32, kind="ExternalOutput")

    with tile.TileContext(nc) as tc:
        _build_body(tc, cfg, L, dram, out_d)
    nc.finalize()
    return nc


def _build_body(tc, cfg, L, dram, out_d):
    from contextlib import ExitStack
    import concourse.mybir as mybir
    f32 = mybir.dt.float32
    AF = mybir.ActivationFunctionType
    OP = mybir.AluOpType
    nc = tc.nc

    NM = cfg["NM"]
    Lm = L // NM                      # megachunk length
    HM = min(1024, Lm)                # psum-tile width for broadcasts
    PC = min(512, HM)                 # matmul N (fp32 max 512)
    CH = min(512, L)                  # prologue chunk
    NCH = L // CH
    DT_S = _dt(cfg["DT_STREAM"])
    DT_A = _dt(cfg["DT_DA"])
    BUFS = cfg["BUFS"]
    DIRS = ("f", "r")
    ROW = {"f": 0, "r": 64}           # partition base per direction
    PAD = DCONV - 1

    ctx = ExitStack()
    with ctx:
        persist = ctx.enter_context(tc.tile_pool(name="persist", bufs=1))

        def wtile(name, shape, dtype=f32):
            t = persist.tile(list(shape), dtype, name=name)
            nc.sync.dma_start(out=t, in_=dram[name][:])
            return t

        toxwT = wtile("toxwT", (DIM, DM))
        toxb = wtile("toxb", (DM, 1))
        toywT = wtile("toywT", (DIM, DIM))
        toyb = wtile("toyb", (DIM, 1))
        projwT = wtile("projwT", (DIM, DIM), DT_S)
        projb = wtile("projb", (DIM, 1))
        selJ2 = wtile("selJ2", (DIM, NT, 128), DT_S)
        P64 = wtile("P64", (DIM, NT, DM), DT_S)
        dtwT2 = wtile("dtwT2", (68, DM))
        convdiag2 = wtile("convdiag2", (DIM, DCONV, DM))
        Ddiag2 = wtile("Ddiag2", (DIM, DM), DT_S)
        W = {}
        for p in DIRS:
            W[p] = {k: wtile(p + k, shp, dt) for k, shp, dt in (
                ("WdtT", (DIM, DTRANK), f32),
                ("WBT", (DIM, NS), f32),
                ("WCT", (DIM, NS), f32),
                ("dtb", (DM, 1), f32),
                ("convb", (DM, 1), f32),
                ("Ascale", (DIM, NT), f32),
            )}

        # ---- persistent activations, f/r packed at partition bases 0/64
        dt2 = persist.tile([128, L], DT_S, name="dt2")
        u2 = persist.tile([128, L], DT_S, name="u2")
        w2 = persist.tile([128, L], DT_S, name="w2")
        B2_sb = {p: persist.tile([128, L], DT_S, name="B2_" + p) for p in DIRS}
        C2_sb = {p: persist.tile([128, L], DT_S, name="C2_" + p) for p in DIRS}
        carry = {p: persist.tile([128, NT], f32, name="carry_" + p) for p in DIRS}
        cat = persist.tile([128, L], DT_S, name="cat")

        def rev_sl(t, row0, row1, c0, n):
            """columns [c0, c0+n) of the time-reversed view, reversed AP"""
            return t[row0:row1, L - c0 - n: L - c0][:, ::-1]

        # ================= prologue =================
        with tc.tile_pool(name="loadp", bufs=1) as loadp, \
             tc.tile_pool(name="ldch", bufs=3) as ldch, \
             tc.tile_pool(name="pps", bufs=4, space="PSUM") as pps, \
             tc.tile_pool(name="prol", bufs=3) as prol:
            xsp2 = loadp.tile([128, L + PAD], f32, name="xsp2")
            dtin2 = loadp.tile([68, L], f32, name="dtin2")
            nc.vector.memset(xsp2[:, 0:PAD], 0.0)

            for c in range(NCH):
                sl = slice(c * CH, (c + 1) * CH)
                Xc = ldch.tile([DIM, CH], f32, name="Xc", tag="Xc")
                nc.sync.dma_start(out=Xc, in_=dram["x"][:, sl])
                xs_ps = pps.tile([DM, CH], f32, name="xs_ps", tag="pps")
                nc.tensor.matmul(xs_ps[:], lhsT=toxwT[:], rhs=Xc[:],
                                 start=True, stop=True)
                nc.scalar.activation(out=xsp2[0:DM, PAD + c * CH: PAD + (c + 1) * CH],
                                     in_=xs_ps[:], func=AF.Identity, bias=toxb[:])
                nc.scalar.activation(
                    out=xsp2[DM:128, PAD + L - (c + 1) * CH: PAD + L - c * CH][:, ::-1],
                    in_=xs_ps[:], func=AF.Identity, bias=toxb[:])

                Yc = ldch.tile([DIM, CH], f32, name="Yc", tag="Yc")
                nc.sync.dma_start(out=Yc, in_=dram["y"][:, sl])
                ys_ps = pps.tile([DIM, CH], f32, name="ys_ps", tag="pps")
                nc.tensor.matmul(ys_ps[:], lhsT=toywT[:], rhs=Yc[:],
                                 start=True, stop=True)
                ysc = ldch.tile([DIM, CH], f32, name="ysc", tag="ysc")
                nc.vector.tensor_scalar_add(out=ysc[:], in0=ys_ps[:],
                                            scalar1=toyb[:])

                for p in DIRS:
                    wp = W[p]
                    B_ps = pps.tile([NS, CH], f32, name="B_ps", tag="pps")
                    nc.tensor.matmul(B_ps[:], lhsT=wp["WBT"][:], rhs=ysc[:],
                                     start=True, stop=True)
                    C_ps = pps.tile([NS, CH], f32, name="C_ps", tag="pps")
                    nc.tensor.matmul(C_ps[:], lhsT=wp["WCT"][:], rhs=ysc[:],
                                     start=True, stop=True)
                    di_ps = pps.tile([DTRANK, CH], f32, name="di_ps", tag="pps")
                    nc.tensor.matmul(di_ps[:], lhsT=wp["WdtT"][:], rhs=ysc[:],
                                     start=True, stop=True)
                    if p == "f":
                        nc.scalar.copy(out=B2_sb[p][0:NS, sl], in_=B_ps[:])
                        nc.scalar.copy(out=B2_sb[p][NS:128, sl], in_=B_ps[:])
                        nc.scalar.copy(out=C2_sb[p][0:NS, sl], in_=C_ps[:])
                        nc.scalar.copy(out=C2_sb[p][NS:128, sl], in_=C_ps[:])
                        nc.vector.tensor_copy(out=dtin2[0:DTRANK, sl], in_=di_ps[:])
                    else:
                        nc.scalar.copy(out=rev_sl(B2_sb[p], 0, NS, c * CH, CH), in_=B_ps[:])
                        nc.scalar.copy(out=rev_sl(B2_sb[p], NS, 128, c * CH, CH), in_=B_ps[:])
                        nc.scalar.copy(out=rev_sl(C2_sb[p], 0, NS, c * CH, CH), in_=C_ps[:])
                        nc.scalar.copy(out=rev_sl(C2_sb[p], NS, 128, c * CH, CH), in_=C_ps[:])
                        nc.vector.tensor_copy(out=dtin2[64:64 + DTRANK, sl], in_=di_ps[:])

            # dt = softplus(dtwT.T @ dtin + dtb) = ln(exp(.) + 1)
            for p in DIRS:
                wp = W[p]
                r0 = ROW[p]
                for c in range(NCH):
                    sl = slice(c * CH, (c + 1) * CH)
                    dt_ps = pps.tile([DM, CH], f32, name="dt_ps", tag="pps")
                    nc.tensor.matmul(dt_ps[:], lhsT=dtwT2[r0:r0 + DTRANK, :],
                                     rhs=dtin2[r0:r0 + DTRANK, sl],
                                     start=True, stop=True)
                    if p == "f":
                        o_ap = dt2[0:DM, sl]
                    else:
                        o_ap = rev_sl(dt2, 64, 128, c * CH, CH)
                    e_t = prol.tile([DM, CH], f32, name="e_t", tag="e_t")
                    nc.scalar.activation(out=e_t[:], in_=dt_ps[:],
                                         func=AF.Exp, bias=wp["dtb"][:])
                    nc.scalar.activation(out=o_ap, in_=e_t[:],
                                         func=AF.Ln, bias=1.0)

            # u = silu(conv(xsp) + convb) = z * sigmoid(z)
            for p in DIRS:
                wp = W[p]
                r0 = ROW[p]
                for c in range(NCH):
                    xc_ps = pps.tile([DM, CH], f32, name="xc_ps", tag="pps")
                    for jj in range(DCONV):
                        nc.tensor.matmul(
                            xc_ps[:], lhsT=convdiag2[r0:r0 + DM, jj, :],
                            rhs=xsp2[r0:r0 + DM, c * CH + jj: c * CH + jj + CH],
                            start=(jj == 0), stop=(jj == DCONV - 1))
                    xcb = prol.tile([DM, CH], f32, name="xcb", tag="xcb")
                    nc.scalar.activation(out=xcb[:], in_=xc_ps[:],
                                         func=AF.Identity, bias=wp["convb"][:])
                    sg = prol.tile([DM, CH], f32, name="sg", tag="sg")
                    nc.scalar.activation(out=sg[:], in_=xc_ps[:],
                                         func=AF.Sigmoid, bias=wp["convb"][:])
                    nc.vector.tensor_mul(u2[r0:r0 + DM, c * CH:(c + 1) * CH],
                                         xcb[:], sg[:])

            for c in range(NCH):
                slc = slice(c * CH, (c + 1) * CH)
                nc.vector.tensor_mul(w2[:, slc], dt2[:, slc], u2[:, slc])

        # ================= scan =================
        SC = min(cfg.get("SC", 2048), Lm)  # l-cols per dtb psum tile
        NHH = Lm // SC
        RC = min(cfg.get("RC", 1024), Lm)  # reduce/output chunk
        NOC = Lm // RC
        TTS_POOL = cfg.get("TTS_POOL", 0)
        with tc.tile_pool(name="bps", bufs=cfg.get("BPS_BUFS", 1), space="PSUM") as bps_pool, \
             tc.tile_pool(name="ops", bufs=cfg.get("OPS_BUFS", 2), space="PSUM") as ops_pool, \
             tc.tile_pool(name="stream", bufs=BUFS) as stream:
            for p in DIRS:
                wp = W[p]
                r0 = ROW[p]
                for m in range(NM):
                    m0 = m * Lm
                    o_ps = []
                    for c in range(NOC):
                        o_t = ops_pool.tile([DM, RC], f32, name="o_ps", tag="o_ps")
                        o_ps.append(o_t)
                    pend = []   # deferred reduce work: (j, hC)
                    RDELAY = cfg.get("RDELAY", 0)

                    def emit_reduce(jj, hCt):
                        emit_reduce2(jj, lambda c0, n: hCt[:, c0:c0 + n])

                    def emit_reduce2(jj, srcf):
                        RQ = max(1, RC // 512)
                        QR = RC // RQ
                        for c in range(NOC):
                            for q in range(RQ):
                                nc.tensor.matmul(
                                    o_ps[c][:, q * QR:(q + 1) * QR], lhsT=P64[:, jj, :],
                                    rhs=srcf(c * RC + q * QR, QR),
                                    start=(jj == 0), stop=False)

                    for j in range(NT):
                        dA = stream.tile([128, Lm], DT_A, name="dA", tag="dA")
                        # w broadcast via DMA (partition-stride-0 source)
                        wbs = stream.tile([128, Lm], DT_S, name="wbs", tag="wbs")
                        NWD = cfg.get("NWD", 2)  # broadcast DMA splits per half
                        WQ = Lm // NWD
                        for wdi in range(NWD):
                            wsl = slice(m0 + wdi * WQ, m0 + (wdi + 1) * WQ)
                            nc.sync.dma_start(
                                out=wbs[0:64, wdi * WQ:(wdi + 1) * WQ],
                                in_=w2[r0 + 2 * j: r0 + 2 * j + 1,
                                       wsl].unsqueeze(1).broadcast_to((1, 64, WQ)))
                            nc.sync.dma_start(
                                out=wbs[64:128, wdi * WQ:(wdi + 1) * WQ],
                                in_=w2[r0 + 2 * j + 1: r0 + 2 * j + 2,
                                       wsl].unsqueeze(1).broadcast_to((1, 64, WQ)))
                        for hh in range(NHH):
                            h0 = hh * SC
                            dtb_ps = bps_pool.tile([128, SC], f32, name="dtb_ps", tag="dtb")
                            NQ = max(1, SC // 512)
                            QW = SC // NQ
                            for q in range(NQ):
                                nc.tensor.matmul(
                                    dtb_ps[:, q * QW:(q + 1) * QW],
                                    lhsT=selJ2[r0:r0 + DM, j, :],
                                    rhs=dt2[r0:r0 + DM, m0 + h0 + q * QW: m0 + h0 + (q + 1) * QW],
                                    start=True, stop=True)
                            nc.scalar.activation(out=dA[:, h0:h0 + SC],
                                                 in_=dtb_ps[:], func=AF.Exp,
                                                 scale=wp["Ascale"][:, j:j + 1])
                        dBu = stream.tile([128, Lm], DT_S, name="dBu", tag="dBu")
                        nc.vector.tensor_mul(dBu[:], wbs[:], B2_sb[p][:, m0:m0 + Lm])
                        h_t = stream.tile([128, Lm], DT_S, name="h_t", tag="h_t")
                        scan_eng = nc.gpsimd if (j % 16) < TTS_POOL else nc.vector
                        scan_eng.tensor_tensor_scan(
                            out=h_t[:], data0=dA[:], data1=dBu[:],
                            initial=(0.0 if m == 0 else carry[p][:, j:j + 1]),
                            op0=OP.mult, op1=OP.add)
                        if m < NM - 1:
                            nc.vector.tensor_copy(out=carry[p][:, j:j + 1],
                                                  in_=h_t[:, Lm - 1:Lm])
                        hc_mode = cfg.get("HC_MODE", "jmod")
                        if hc_mode == "twotile":
                            HS = cfg.get("HSPLIT", 1536)
                            hCa = stream.tile([128, HS], DT_S, name="hCa", tag="hCa")
                            nc.gpsimd.tensor_mul(hCa[:], h_t[:, 0:HS],
                                                 C2_sb[p][:, m0:m0 + HS])
                            hCb = stream.tile([128, Lm - HS], DT_S, name="hCb", tag="hCb")
                            nc.vector.tensor_mul(hCb[:], h_t[:, HS:Lm],
                                                 C2_sb[p][:, m0 + HS:m0 + Lm])

                            def hc_src(c0, n):
                                """hC view for scan-cols [c0, c0+n) (within one tile)"""
                                if c0 + n <= HS:
                                    return hCa[:, c0:c0 + n]
                                return hCb[:, c0 - HS:c0 - HS + n]
                            pend.append((j, hc_src))
                            if len(pend) > RDELAY:
                                jj, srcf = pend.pop(0)
                                emit_reduce2(jj, srcf)
                            continue
                        hC = stream.tile([128, Lm], DT_S, name="hC", tag="hC")
                        if hc_mode == "jmod":
                            on_pool = (j % 6) < cfg.get("HC_POOL", 0)
                        elif hc_mode == "mega":
                            on_pool = (m == 0)
                        elif hc_mode == "dir":
                            on_pool = (p == "f")
                        elif hc_mode == "tail":
                            on_pool = j < NT - cfg.get("HC_TAIL", 4)
                        elif hc_mode == "dvesplit":
                            # halves of the same hC tile on both engines
                            on_pool = None
                        else:
                            on_pool = True
                        if on_pool is None:
                            HSP = cfg.get("HSPLIT", Lm // 2)
                            nc.gpsimd.tensor_mul(hC[:, 0:HSP], h_t[:, 0:HSP],
                                                 C2_sb[p][:, m0:m0 + HSP])
                            nc.vector.tensor_mul(hC[:, HSP:Lm], h_t[:, HSP:Lm],
                                                 C2_sb[p][:, m0 + HSP:m0 + Lm])
                        elif on_pool:
                            nc.gpsimd.tensor_mul(hC[:], h_t[:], C2_sb[p][:, m0:m0 + Lm])
                        else:
                            hC = stream.tile([128, Lm], DT_S, name="hCd", tag="hCd")
                            nc.vector.tensor_mul(hC[:], h_t[:], C2_sb[p][:, m0:m0 + Lm])
                        pend.append((j, hC))
                        if len(pend) > RDELAY:
                            jj, hCt = pend.pop(0)
                            emit_reduce(jj, hCt)
                    for jj, item in pend:
                        if callable(item):
                            emit_reduce2(jj, item)
                        else:
                            emit_reduce(jj, item)
                    RQ = max(1, RC // 512)
                    QR = RC // RQ
                    for c in range(NOC):
                        for q in range(RQ):
                            nc.tensor.matmul(
                                o_ps[c][:, q * QR:(q + 1) * QR], lhsT=Ddiag2[r0:r0 + DM, :],
                                rhs=u2[r0:r0 + DM, m0 + c * RC + q * QR: m0 + c * RC + (q + 1) * QR],
                                start=False, stop=True)
                        if p == "f":
                            nc.scalar.copy(out=cat[0:DM, m0 + c * RC: m0 + (c + 1) * RC],
                                           in_=o_ps[c][:])
                        else:
                            nc.scalar.copy(out=rev_sl(cat, DM, 128, m0 + c * RC, RC),
                                           in_=o_ps[c][:])

        # ================= proj =================
        CP = min(1024, L)
        with tc.tile_pool(name="prps", bufs=2, space="PSUM") as prps, \
             tc.tile_pool(name="stage", bufs=3) as stage:
            for c in range(L // CP):
                sl = slice(c * CP, (c + 1) * CP)
                pr_ps = prps.tile([DIM, CP], f32, name="pr_ps")
                PQ = CP // 512 if CP >= 512 else 1
                QP = CP // PQ
                for q in range(PQ):
                    nc.tensor.matmul(
                        pr_ps[:, q * QP:(q + 1) * QP], lhsT=projwT[:],
                        rhs=cat[:, c * CP + q * QP: c * CP + (q + 1) * QP],
                        start=True, stop=True)
                st = stage.tile([DIM, CP], f32, name="st")
                nc.scalar.activation(out=st[:], in_=pr_ps[:],
                                     func=AF.Identity, bias=projb[:])
                nc.sync.dma_start(out=out_d[:, sl], in_=st[:])


# ---------------------------------------------------------------- host side

def host_weights(inputs, cfg):
    """Per-core weight arrays (shared across cores) from the raw inputs."""
    np_s = _np_dt(cfg["DT_STREAM"])
    wmap = {
        "toxwT": np.ascontiguousarray(np.asarray(inputs["to_x_w"], np.float32).T),
        "toxb": np.asarray(inputs["to_x_b"], np.float32).reshape(DM, 1).copy(),
        "toywT": np.ascontiguousarray(np.asarray(inputs["to_y_w"], np.float32).T),
        "toyb": np.asarray(inputs["to_y_b"], np.float32).reshape(DIM, 1).copy(),
        "projwT": np.ascontiguousarray(np.asarray(inputs["proj_w"], np.float32).T).astype(np_s),
        "projb": np.asarray(inputs["proj_b"], np.float32).reshape(DIM, 1).copy(),
    }
    selJ2 = np.zeros((DIM, NT, 128), np.float32)
    P64 = np.zeros((DIM, NT, DM), np.float32)
    for j in range(NT):
        for half in (0, 64):
            selJ2[half + 2 * j, j, 0:64] = 1.0
            selJ2[half + 2 * j + 1, j, 64:128] = 1.0
        P64[0:64, j, 2 * j] = 1.0
        P64[64:128, j, 2 * j + 1] = 1.0
    wmap["selJ2"] = selJ2.astype(np_s)
    wmap["P64"] = P64.astype(np_s)

    dtwT2 = np.zeros((68, DM), np.float32)
    convdiag2 = np.zeros((DIM, DCONV, DM), np.float32)
    Ddiag2 = np.zeros((DIM, DM), np.float32)
    for p, base in (("f", 0), ("r", 64)):
        pre = p + "_"
        yproj = np.asarray(inputs[pre + "yproj_w"], np.float32)
        wmap[p + "WdtT"] = np.ascontiguousarray(yproj[0:DTRANK].T)
        wmap[p + "WBT"] = np.ascontiguousarray(yproj[DTRANK:DTRANK + NS].T)
        wmap[p + "WCT"] = np.ascontiguousarray(yproj[DTRANK + NS:].T)
        dtwT2[base:base + DTRANK] = np.asarray(inputs[pre + "dt_w"], np.float32).T
        wmap[p + "dtb"] = np.asarray(inputs[pre + "dt_b"], np.float32).reshape(DM, 1).copy()
        conv_w = np.asarray(inputs[pre + "conv_w"], np.float32)
        for jj in range(DCONV):
            convdiag2[base + np.arange(DM), jj, np.arange(DM)] = conv_w[:, jj]
        wmap[p + "convb"] = np.asarray(inputs[pre + "conv_b"], np.float32).reshape(DM, 1).copy()
        A = -np.exp(np.asarray(inputs[pre + "A_log"], np.float64)).astype(np.float32)
        Ascale = np.zeros((DIM, NT), np.float32)
        for j in range(NT):
            Ascale[0:64, j] = A[2 * j, :]
            Ascale[64:128, j] = A[2 * j + 1, :]
        wmap[p + "Ascale"] = Ascale
        Ddiag2[base + np.arange(DM), np.arange(DM)] = np.asarray(inputs[pre + "D"], np.float32)
    wmap["dtwT2"] = dtwT2
    wmap["convdiag2"] = convdiag2
    wmap["Ddiag2"] = Ddiag2.astype(np_s)
    return wmap


def kernel(**inputs):
    from concourse.bass_utils import run_bass_kernel_spmd

    cfg = CFG
    key = str(sorted(cfg.items()))
    if key not in _BUILD_CACHE:
        _BUILD_CACHE[key] = build_program(cfg)
    nc = _BUILD_CACHE[key]

    wmap = host_weights(inputs, cfg)
    x = np.asarray(inputs["x"], np.float32).reshape(B_TOT, DIM, L)
    y = np.asarray(inputs["y"], np.float32).reshape(B_TOT, DIM, L)

    in_maps = []
    for b in range(B_TOT):
        m = dict(wmap)
        m["x"] = np.ascontiguousarray(x[b])
        m["y"] = np.ascontiguousarray(y[b])
        in_maps.append(m)

    global LAST_RESULT
    res = run_bass_kernel_spmd(nc, in_maps, core_ids=list(range(B_TOT)))
    LAST_RESULT = res
    out = np.stack([res.results[b]["out"] for b in range(B_TOT)], axis=0)
    return out.reshape(B_TOT, DIM, H, W).astype(np.float32)
